# revision 1
# baseline (speedup 1.0000x reference)
"""LiquidCell Trainium2 kernel (Bass/Tile, 8-core SPMD, data-parallel over batch).

Reference computation (B=4096, I=1024, H=2048, 5 steps):
    input_contrib = x @ W_in_w.T + W_in_b
    x_tau = x @ tau_adapt_w[:, :I].T
    h = hidden
    for _ in range(5):
        tau_logits = x_tau + h @ tau_adapt_w[:, I:].T + tau_adapt_b
        tau = tau_base * (0.5 + sigmoid(tau_logits))
        activated = tanh(h @ W_rec.T + input_contrib)
        h = h + DT * (-h + activated) / tau
    return (h, tau)

Strategy: shard batch across 8 cores (512 rows each), replicate weights.
On-chip everything is feature-major ([features on partitions, batch cols free])
so the recurrent state h feeds matmuls without transposes; all transposes
happen on host. Matmuls run in float32r (TF32-like, full PE rate at N=512,
~2e-4 relative error per matmul), accumulating fp32 in PSUM. Loop-invariant
x-side contributions are computed once on chip and added to each step's PSUM
result on the vector engine. Weight matrices for the recurrent matmuls
(16 MiB each) stream from HBM per step, double buffered on the two HWDGE
rings, fully under the PE shadow.
"""

import os

import numpy as np

import concourse.bacc as bacc
import concourse.mybir as mybir
import concourse.tile as tile
from concourse.bass_utils import run_bass_kernel_spmd

F32 = mybir.dt.float32
F32R = mybir.dt.float32r
AF = mybir.ActivationFunctionType
ALU = mybir.AluOpType

B, I, H = 4096, 1024, 2048
NUM_STEPS = 5
DT = 0.1
NCORES = 8
BL = B // NCORES          # 512 batch rows per core
P = 128
JT = H // P               # 16 output-feature tiles
KTH = H // P              # 16 contraction tiles (h side)
KTX = I // P              # 8 contraction tiles (x side)

# exposed for test harness (set when BASS_TRACE=1)
LAST_EXEC_NS = None


def _build():
    nc = bacc.Bacc()
    xT_d = nc.declare_dram_parameter("xT", [I, BL], F32R, isOutput=False)
    hT_d = nc.declare_dram_parameter("hT", [H, BL], F32R, isOutput=False)
    Wr_d = nc.declare_dram_parameter("Wr", [JT, P, KTH, P], F32R, isOutput=False)
    Th_d = nc.declare_dram_parameter("Th", [JT, P, KTH, P], F32R, isOutput=False)
    Wi_d = nc.declare_dram_parameter("Wi", [JT, P, KTX, P], F32R, isOutput=False)
    Tx_d = nc.declare_dram_parameter("Tx", [JT, P, KTX, P], F32R, isOutput=False)
    # per-feature vectors, laid out [P, JT] (col j = features j*128..j*128+127)
    taub_d = nc.declare_dram_parameter("taub", [P, JT], F32, isOutput=False)
    tb_d = nc.declare_dram_parameter("tb", [P, JT], F32, isOutput=False)
    htb_d = nc.declare_dram_parameter("htb", [P, JT], F32, isOutput=False)
    winb_d = nc.declare_dram_parameter("winb", [P, JT], F32, isOutput=False)
    hout_d = nc.declare_dram_parameter("hout", [H, BL], F32R, isOutput=True)
    tauout_d = nc.declare_dram_parameter("tauout", [H, BL], F32, isOutput=True)

    with tile.TileContext(nc) as tc:
        with tc.tile_pool(name="const", bufs=1) as const, \
             tc.tile_pool(name="state", bufs=2) as state, \
             tc.tile_pool(name="xside", bufs=1) as xside, \
             tc.tile_pool(name="wstream", bufs=2) as wstream, \
             tc.tile_pool(name="wpre", bufs=3) as wpre, \
             tc.tile_pool(name="sc", bufs=2) as sc, \
             tc.tile_pool(name="ps", bufs=4, space="PSUM") as ps:

            rings = (nc.scalar, nc.sync)
            # Cold-start order matters: the first preamble group needs its
            # weight slab and the first xT tiles before anything else. Only
            # the two HWDGE rings carry latency-critical transfers — the
            # gpsimd SWDGE path pays a Q7 library-load + drain at start that
            # gates its first transfers by >10us.
            pre_slabs = []
            xT = state.tile([P, KTX, BL], F32R, tag="state")
            txs = wpre.tile([P, KTX, P], F32R, tag="tx")
            nc.scalar.dma_start(out=txs, in_=Tx_d[0])
            wis = wpre.tile([P, KTX, P], F32R, tag="wi")
            nc.sync.dma_start(out=wis, in_=Wi_d[0])
            pre_slabs.append((txs, wis))
            for k in range(KTX):
                rings[k % 2].dma_start(out=xT[:, k, :], in_=xT_d[k * P:(k + 1) * P, :])
            for j in range(1, 3):
                txs = wpre.tile([P, KTX, P], F32R, tag="tx")
                nc.scalar.dma_start(out=txs, in_=Tx_d[j])
                wis = wpre.tile([P, KTX, P], F32R, tag="wi")
                nc.sync.dma_start(out=wis, in_=Wi_d[j])
                pre_slabs.append((txs, wis))
            taub = const.tile([P, JT], F32)
            nc.gpsimd.dma_start(out=taub, in_=taub_d[:])
            tb = const.tile([P, JT], F32)
            nc.gpsimd.dma_start(out=tb, in_=tb_d[:])
            htb = const.tile([P, JT], F32)
            nc.gpsimd.dma_start(out=htb, in_=htb_d[:])
            winb = const.tile([P, JT], F32)
            nc.gpsimd.dma_start(out=winb, in_=winb_d[:])
            # h state rides the gpsimd SWDGE ring: it is not consumed until
            # the first recurrent step (~58us in), so the Q7 startup cost and
            # the transfer itself hide entirely under the preamble, keeping
            # the HWDGE rings free for weight-slab prefetch.
            h_cur = state.tile([P, KTH, BL], F32R, tag="state")
            for k in range(KTH):
                nc.gpsimd.dma_start(out=h_cur[:, k, :], in_=hT_d[k * P:(k + 1) * P, :])

            x_tau = xside.tile([P, JT, BL], F32)
            ic = xside.tile([P, JT, BL], F32)

            # ---- preamble (x-side matmuls) interleaved with step 0 so the
            # PE always has runnable work while the DMA rings warm up ----
            def preamble_j(j):
                if j < 3:
                    txs, wis = pre_slabs[j]
                else:
                    txs = wpre.tile([P, KTX, P], F32R, tag="tx")
                    nc.scalar.dma_start(out=txs, in_=Tx_d[j])
                    wis = wpre.tile([P, KTX, P], F32R, tag="wi")
                    nc.sync.dma_start(out=wis, in_=Wi_d[j])
                pt = ps.tile([P, BL], F32, tag="pt")
                for k in range(KTX):
                    nc.tensor.matmul(pt, txs[:, k, :], xT[:, k, :],
                                     start=(k == 0), stop=(k == KTX - 1))
                nc.scalar.activation(x_tau[:, j, :], pt, AF.Copy)
                pr = ps.tile([P, BL], F32, tag="pr")
                for k in range(KTX):
                    nc.tensor.matmul(pr, wis[:, k, :], xT[:, k, :],
                                     start=(k == 0), stop=(k == KTX - 1))
                nc.scalar.activation(ic[:, j, :], pr, AF.Identity,
                                     bias=winb[:, j:j + 1])

            def step_j(step, j, h_cur, h_nxt):
                last = step == NUM_STEPS - 1
                ths = wstream.tile([P, KTH, P], F32R, tag="th")
                nc.scalar.dma_start(out=ths, in_=Th_d[j])
                wrs = wstream.tile([P, KTH, P], F32R, tag="wr")
                nc.sync.dma_start(out=wrs, in_=Wr_d[j])

                pt = ps.tile([P, BL], F32, tag="pt")
                for k in range(KTH):
                    nc.tensor.matmul(pt, ths[:, k, :], h_cur[:, k, :],
                                     start=(k == 0), stop=(k == KTH - 1))
                pr = ps.tile([P, BL], F32, tag="pr")
                for k in range(KTH):
                    nc.tensor.matmul(pr, wrs[:, k, :], h_cur[:, k, :],
                                     start=(k == 0), stop=(k == KTH - 1))

                lg = sc.tile([P, BL], F32, tag="e3")
                nc.vector.tensor_tensor(out=lg, in0=pt, in1=x_tau[:, j, :],
                                        op=ALU.add)
                s_ = sc.tile([P, BL], F32, tag="s")
                nc.scalar.activation(s_, lg, AF.Sigmoid, bias=taub[:, j:j + 1])
                tau = sc.tile([P, BL], F32, tag="tau")
                nc.scalar.activation(tau, s_, AF.Identity,
                                     bias=htb[:, j:j + 1], scale=tb[:, j:j + 1])
                q = sc.tile([P, BL], F32, tag="s")
                nc.vector.reciprocal_approx_fast(out=q, in_=tau)

                pre = sc.tile([P, BL], F32, tag="e3")
                nc.vector.tensor_tensor(out=pre, in0=pr, in1=ic[:, j, :],
                                        op=ALU.add)
                a = sc.tile([P, BL], F32, tag="e3")
                nc.scalar.activation(a, pre, AF.Tanh)
                d = sc.tile([P, BL], F32, tag="du")
                nc.vector.tensor_tensor(out=d, in0=a, in1=h_cur[:, j, :],
                                        op=ALU.subtract)
                u = sc.tile([P, BL], F32, tag="du")
                nc.vector.scalar_tensor_tensor(out=u, in0=d, scalar=DT, in1=q,
                                               op0=ALU.mult, op1=ALU.mult)
                nc.vector.tensor_tensor(out=h_nxt[:, j, :], in0=u,
                                        in1=h_cur[:, j, :], op=ALU.add)
                if last:
                    # gpsimd ring is idle after the initial h load; keeping
                    # outputs off the HWDGE rings protects the last step's
                    # weight-slab prefetch
                    nc.gpsimd.dma_start(out=hout_d[j * P:(j + 1) * P, :],
                                        in_=h_nxt[:, j, :])
                    nc.gpsimd.dma_start(out=tauout_d[j * P:(j + 1) * P, :],
                                        in_=tau)

            for j in range(JT):
                preamble_j(j)
            for step in range(NUM_STEPS):
                h_nxt = state.tile([P, KTH, BL], F32R, tag="state")
                for j in range(JT):
                    step_j(step, j, h_cur, h_nxt)
                h_cur = h_nxt
    nc.finalize()
    return nc


_NC_CACHE = None


def _get_nc():
    global _NC_CACHE
    if _NC_CACHE is None:
        _NC_CACHE = _build()
    return _NC_CACHE


def _prep_w(W):
    """W [J, K] row-major -> [jt, p, kt, c] with element [jt,p,kt,c] = W[jt*P+c, kt*P+p]."""
    J, K = W.shape
    ktn = K // P
    jtn = J // P
    Bv = np.ascontiguousarray(W.T).reshape(ktn, P, jtn, P)
    return np.ascontiguousarray(Bv.transpose(2, 1, 0, 3))


def _prep_vec(v):
    """[H] -> [P, JT] with col j = v[j*128:(j+1)*128]."""
    return np.ascontiguousarray(np.asarray(v, np.float32).reshape(JT, P).T)


def kernel(x, hidden, W_rec, W_in_w, W_in_b, tau_base, tau_adapt_w, tau_adapt_b):
    global LAST_EXEC_NS
    x = np.asarray(x, np.float32)
    hidden = np.asarray(hidden, np.float32)
    W_rec = np.asarray(W_rec, np.float32)
    W_in_w = np.asarray(W_in_w, np.float32)
    tau_adapt_w = np.asarray(tau_adapt_w, np.float32)

    shared = {
        "Wr": _prep_w(W_rec),
        "Th": _prep_w(tau_adapt_w[:, I:]),
        "Wi": _prep_w(W_in_w),
        "Tx": _prep_w(tau_adapt_w[:, :I]),
        "taub": _prep_vec(tau_adapt_b),
        "tb": _prep_vec(tau_base),
        "htb": _prep_vec(np.asarray(tau_base, np.float32) * 0.5),
        "winb": _prep_vec(W_in_b),
    }
    in_maps = []
    for c in range(NCORES):
        sl = slice(c * BL, (c + 1) * BL)
        in_maps.append(dict(shared,
                            xT=np.ascontiguousarray(x[sl].T),
                            hT=np.ascontiguousarray(hidden[sl].T)))

    nc = _get_nc()
    trace = bool(os.environ.get("BASS_TRACE"))
    res = None
    for attempt in range(3):
        try:
            res = run_bass_kernel_spmd(nc, in_maps, list(range(NCORES)), trace=trace)
            break
        except Exception:
            # transient device errors (NRT unrecoverable) clear on retry
            # after the runtime resets the core
            if attempt == 2:
                raise

    if trace:
        LAST_EXEC_NS = res.exec_time_ns

    h_out = np.concatenate(
        [np.ascontiguousarray(res.results[c]["hout"].T) for c in range(NCORES)], axis=0)
    tau_out = np.concatenate(
        [np.ascontiguousarray(res.results[c]["tauout"].T) for c in range(NCORES)], axis=0)
    return h_out, tau_out

